# revision 1
# baseline (speedup 1.0000x reference)
"""NeuralKB retrieval kernel v4: v3 + software-pipelined prologue.

Same math/sharding as v3 (score x entity sharding, one score per core,
1024 entities, f-on-partitions). Difference: the per-group prologue
(fact transposes+casts, f2, W columns) for group g+1 is emitted in
pieces BETWEEN the stage-1 chunks of group g, so the in-order engines
overlap it with steady-state work instead of stalling at group
boundaries. Input DMAs are spread across the SP/ACT/DVE hardware DGE
queues so the three fact tensors load in parallel.

Piece schedule (8 pieces per group, quarter-group granularity):
  piece p (p=0..7): transposes+casts for chunk p of the group;
  after odd p: squares + f2 + W matmuls + Wcopy/Weff for quarter p//2.
Stage-1 chunk c of group g is emitted after piece (c|1)+... see code.
"""

import contextlib

import numpy as np

import concourse.bass as bass
import concourse.tile as tile
from concourse import bacc, mybir
from concourse import bass_utils
from concourse.masks import make_identity
from concourse.bass_isa import ReduceOp

F32 = mybir.dt.float32
BF16 = mybir.dt.bfloat16
AF = mybir.ActivationFunctionType
ALU = mybir.AluOpType

B = 8
E = 100
F = 4000
FP = 4096
NCHUNK = FP // 128
GROUPS = 4
GCH = NCHUNK // GROUPS  # 8
NN = 1024
NHALF = NN // 2
XW = B * NN

DVE_SLOTS = (4, 5, 6, 7)
ACT_SLOTS = (1, 2, 3)
POOL_SLOTS = ()
KSPLIT = 8192


def build_bass(repeat=1, ksplit=KSPLIT, dve_slots=DVE_SLOTS, act_slots=ACT_SLOTS,
               pool_slots=POOL_SLOTS, debug=False, scope="full"):
    assert sorted((0,) + tuple(dve_slots) + tuple(act_slots) + tuple(pool_slots)) == list(range(B))
    nc = bacc.Bacc("TRN2", target_bir_lowering=False, debug=False, num_devices=8)

    f_c = nc.dram_tensor("f_c", [FP, E], F32, kind="ExternalInput")
    f_w = nc.dram_tensor("f_w", [FP, E], F32, kind="ExternalInput")
    f_rel = nc.dram_tensor("f_rel", [FP, E], F32, kind="ExternalInput")
    ent = nc.dram_tensor("ent", [NN, E], F32, kind="ExternalInput")
    rel = nc.dram_tensor("rel", [B, E], F32, kind="ExternalInput")
    qw = nc.dram_tensor("qw", [B, E], F32, kind="ExternalInput")
    out = nc.dram_tensor("out", [B, NN], F32, kind="ExternalOutput")

    with tile.TileContext(nc) as tc:
        with (
            tc.tile_pool(name="const", bufs=1) as const_pool,
            tc.tile_pool(name="factT", bufs=1) as factT_pool,
            tc.tile_pool(name="acc", bufs=1) as acc_pool,
            tc.tile_pool(name="small", bufs=1) as small_pool,
            tc.tile_pool(name="nat", bufs=1) as nat_pool,
            tc.tile_pool(name="sq", bufs=2) as sq_pool,
            tc.tile_pool(name="xall", bufs=3) as xall_pool,
            tc.tile_pool(name="fin", bufs=1) as fin_pool,
            tc.tile_pool(name="tpsum", bufs=2, space="PSUM") as tpsum_pool,
            tc.tile_pool(name="cpsum", bufs=2, space="PSUM") as cpsum_pool,
            tc.tile_pool(name="wpsum", bufs=1, space="PSUM") as wpsum_pool,
        ):
            pools = (const_pool, factT_pool, acc_pool, small_pool, nat_pool,
                     sq_pool, xall_pool, fin_pool, tpsum_pool, cpsum_pool,
                     wpsum_pool)

            ident = const_pool.tile([128, 128], F32, tag="ident")
            make_identity(nc, ident[:])
            frelT = factT_pool.tile([101, FP], BF16, tag="frelT")
            fwT = factT_pool.tile([101, FP], BF16, tag="fwT")
            fcT = factT_pool.tile([100, FP], BF16, tag="fcT")
            onesrow = small_pool.tile([1, FP], BF16, tag="onesrow")
            nc.gpsimd.memset(onesrow[:], 1.0)
            nc.sync.dma_start(frelT[100:101, :], onesrow[:])
            relmov = const_pool.tile([101, B], BF16, tag="relmov")
            qwmov = const_pool.tile([101, B], BF16, tag="qwmov")
            negrow = small_pool.tile([1, B], BF16, tag="negrow")
            nc.gpsimd.memset(negrow[:], -1.0)
            nc.sync.dma_start(qwmov[100:101, :], negrow[:])
            ones_col = const_pool.tile([100, 1], BF16, tag="ones_col")
            nc.gpsimd.memset(ones_col[:], 1.0)
            statics = (ident, frelT, fwT, fcT, relmov, qwmov, ones_col)

            rep_ctx = tc.For_i(0, repeat, 1) if repeat > 1 else contextlib.nullcontext()
            if scope == "full":
                with rep_ctx:
                    _full_body(nc, tc, pools, statics, f_c, f_w, f_rel, ent,
                               rel, qw, out, ksplit, dve_slots, act_slots,
                               pool_slots, None)
            else:
                _full_body(nc, tc, pools, statics, f_c, f_w, f_rel, ent, rel,
                           qw, out, ksplit, dve_slots, act_slots, pool_slots,
                           rep_ctx)
    nc.compile()
    return nc


def _full_body(nc, tc, pools, statics, f_c, f_w, f_rel, ent, rel, qw, out,
               ksplit, dve_slots, act_slots, pool_slots, stage1_ctx):
    (const_pool, factT_pool, acc_pool, small_pool, nat_pool, sq_pool,
     xall_pool, fin_pool, tpsum_pool, cpsum_pool, wpsum_pool) = pools
    (ident, frelT, fwT, fcT, relmov, qwmov, ones_col) = statics

    # ---------------- input loads (parallel DGE queues) ----------------------
    nats = []
    dma_engines = (nc.sync, nc.scalar, nc.sync)
    for i, dram in enumerate((f_rel, f_w, f_c)):
        natt = nat_pool.tile([128, NCHUNK * E], F32, tag=f"nat_{i}")
        nats.append(natt)

    def load_chunks(c0, c1):
        for i, dram in enumerate((f_rel, f_w, f_c)):
            dma_engines[i].dma_start(
                nats[i][:, c0 * E : c1 * E].rearrange("p (c e) -> p c e", e=E),
                dram.ap()[c0 * 128 : c1 * 128, :].rearrange(
                    "(c p) e -> p c e", p=128
                ),
            )

    qts = {}
    for name, dram in (("rel", rel), ("qw", qw)):
        qt = small_pool.tile([B, E], F32, tag=f"q_{name}")
        nc.sync.dma_start(qt[:], dram.ap())
        qts[name] = qt
    entn = nat_pool.tile([128, 8 * E], F32, tag="entn")
    nc.sync.dma_start(
        entn[:].rearrange("p (c e) -> p c e", e=E),
        ent.ap().rearrange("(c p) e -> p c e", p=128),
    )
    load_chunks(16, 32)
    load_chunks(0, 4)
    load_chunks(4, 16)

    # ---------------- queries ------------------------------------------------
    for name, dst in (("rel", relmov), ("qw", qwmov)):
        tp = tpsum_pool.tile([128, NHALF], F32, tag="tp")
        nc.tensor.transpose(tp[:E, :B], qts[name][:], ident[:B, :B])
        nc.scalar.activation(dst[0:100, :], tp[:E, :B], AF.Copy, scale=2.0)
    sqs = {}
    for name in ("rel", "qw"):
        sq = small_pool.tile([B, E], F32, tag=f"qsq_{name}")
        nc.scalar.activation(sq[:], qts[name][:], AF.Square)
        r = small_pool.tile([B, 1], F32, tag=f"qr_{name}")
        nc.vector.tensor_reduce(r[:], sq[:], axis=mybir.AxisListType.X, op=ALU.add)
        sqs[name] = r
    q2 = small_pool.tile([B, 1], F32, tag="q2")
    nc.vector.tensor_tensor(q2[:], sqs["rel"][:], sqs["qw"][:], op=ALU.add)
    q2tp = tpsum_pool.tile([128, NHALF], F32, tag="tp")
    nc.tensor.transpose(q2tp[:1, :B], q2[:], ident[:B, :B])
    q2neg = small_pool.tile([1, B], BF16, tag="q2neg")
    nc.scalar.activation(q2neg[:], q2tp[:1, :B], AF.Copy, scale=-1.0)
    nc.sync.dma_start(relmov[100:101, :], q2neg[:])

    # ---------------- entities ----------------------------------------------
    ent2T = const_pool.tile([100, NN], BF16, tag="ent2T")
    for c in range(8):
        tp = tpsum_pool.tile([128, NHALF], F32, tag="tp")
        nc.tensor.transpose(tp[:100, :128], entn[:, c * E : (c + 1) * E], ident[:])
        if c % 2 == 0:
            nc.vector.tensor_scalar(
                out=ent2T[:, c * 128 : (c + 1) * 128], in0=tp[:100, :128],
                scalar1=2.0, scalar2=None, op0=ALU.mult)
        else:
            nc.scalar.activation(
                ent2T[:, c * 128 : (c + 1) * 128], tp[:100, :128], AF.Copy,
                scale=2.0)
    e2row = small_pool.tile([1, NN], F32, tag="e2row")

    def emit_e2():
        entsqT = sq_pool.tile([100, NN], BF16, tag="entsqT")
        nc.vector.tensor_tensor(entsqT[:], ent2T[:], ent2T[:], op=ALU.mult)
        for h in range(2):
            hs = slice(h * NHALF, (h + 1) * NHALF)
            e2p = tpsum_pool.tile([128, NHALF], F32, tag="tp")
            nc.tensor.matmul(e2p[:1, :], ones_col[:], entsqT[:, hs], start=True,
                             stop=True)
            nc.scalar.activation(e2row[:, hs], e2p[:1, :], AF.Copy, scale=0.25)

    wpsum = wpsum_pool.tile([128, 512], F32, tag="wpsum")
    W_sb = const_pool.tile([128, NCHUNK * B], F32, tag="W_sb")
    Weff = const_pool.tile([128, NCHUNK * B], F32, tag="Weff")
    acc_all = acc_pool.tile([128, XW], BF16, tag="acc_all")

    # ---------------- pipelined prologue pieces + stage-1 --------------------
    def tcasts_quad(c0):
        """PE transposes for chunks c0..c0+3 + one wide cast per tensor.

        For the first two groups (the ramp, where DVE is idle and ACT's
        serial queue gates the first carrier) one cast per quad goes to
        DVE; later groups keep DVE free for steady-state work."""
        for i, dstT in enumerate((frelT, fwT, fcT)):
            tp = tpsum_pool.tile([128, NHALF], F32, tag="tp")
            for k in range(4):
                c = c0 + k
                ces = slice(c * E, (c + 1) * E)
                nc.tensor.transpose(tp[:100, k * 128 : (k + 1) * 128],
                                    nats[i][:, ces], ident[:])
            if c0 < 16 and i == 0:
                nc.vector.tensor_scalar(
                    out=dstT[0:100, c0 * 128 : (c0 + 4) * 128],
                    in0=tp[:100, :], scalar1=1.0, scalar2=None, op0=ALU.mult)
            else:
                nc.scalar.activation(
                    dstT[0:100, c0 * 128 : (c0 + 4) * 128], tp[:100, :],
                    AF.Copy)

    def group_fw(g):
        """f2 + W columns for the 8 chunks of group g."""
        gs = slice(g * GCH * 128, (g + 1) * GCH * 128)  # 1024 f cols
        sqg = sq_pool.tile([100, 3 * 1024], BF16, tag="sqg")
        for i, srcT in enumerate((frelT, fwT, fcT)):
            nc.scalar.activation(sqg[:, i * 1024 : (i + 1) * 1024],
                                 srcT[0:100, gs], AF.Square)
        f2st = sq_pool.tile([1, 1024], BF16, tag="f2st")
        for h in range(2):
            f2p = tpsum_pool.tile([128, NHALF], F32, tag="tp")
            for i in range(3):
                nc.tensor.matmul(
                    f2p[:1, 0:512], ones_col[:],
                    sqg[:, i * 1024 + h * 512 : i * 1024 + (h + 1) * 512],
                    start=(i == 0), stop=(i == 2))
            nc.scalar.activation(f2st[:, h * 512 : (h + 1) * 512],
                                 f2p[:1, 0:512], AF.Copy)
        nc.sync.dma_start(fwT[100:101, gs], f2st[:])
        for c in range(g * GCH, (g + 1) * GCH):
            cs = slice(c * 128, (c + 1) * 128)
            ws = slice(c * B, (c + 1) * B)
            nc.tensor.matmul(wpsum[:, ws], frelT[:, cs], relmov[:],
                             start=True, stop=False)
            nc.tensor.matmul(wpsum[:, ws], fwT[:, cs], qwmov[:],
                             start=False, stop=True)
        gws = slice(g * GCH * B, (g + 1) * GCH * B)
        nc.scalar.activation(W_sb[:, gws], wpsum[:, gws], AF.Copy)
        wv = W_sb[:, gws].rearrange("p (c s) -> p c s", s=B)
        ev = Weff[:, gws].rearrange("p (c s) -> p c s", s=B)
        nc.vector.tensor_tensor(
            ev[:, :, 1:B], wv[:, :, 1:B],
            wv[:, :, 0:1].broadcast_to([128, GCH, B - 1]), op=ALU.subtract)

    def stage1(c):
        cs = slice(c * 128, (c + 1) * 128)
        wcol = lambda s: W_sb[:, c * B + s : c * B + s + 1]
        dcol = lambda s: Weff[:, c * B + s : c * B + s + 1]
        cp = cpsum_pool.tile([128, NN], F32, tag="cp")
        nc.tensor.matmul(cp[:, 0:NHALF], fcT[0:100, cs], ent2T[:, 0:NHALF],
                         start=True, stop=True)
        nc.tensor.matmul(cp[:, NHALF:NN], fcT[0:100, cs], ent2T[:, NHALF:NN],
                         start=True, stop=True)
        xt = acc_all if c == 0 else xall_pool.tile([128, XW], BF16, tag="xall")
        xsec = lambda s: xt[:, s * NN : (s + 1) * NN]
        carrier = xsec(0)
        nc.scalar.activation(carrier, cp[:], AF.Identity, bias=wcol(0))
        for s in act_slots:
            nc.scalar.activation(xsec(s), cp[:], AF.Identity, bias=wcol(s))
        for s in dve_slots:
            nc.vector.tensor_scalar(
                out=xsec(s), in0=carrier, scalar1=dcol(s), scalar2=None,
                op0=ALU.add)
        for s in pool_slots:
            nc.gpsimd.tensor_scalar(
                out=xsec(s), in0=carrier, scalar1=dcol(s), scalar2=None,
                op0=ALU.add)
        if c > 0:
            if c == NCHUNK - 1:
                nc.vector.tensor_tensor(acc_all[:, 0 : XW // 2],
                                        acc_all[:, 0 : XW // 2],
                                        xt[:, 0 : XW // 2], op=ALU.max)
                nc.vector.tensor_tensor(acc_all[:, XW // 2 :],
                                        acc_all[:, XW // 2 :],
                                        xt[:, XW // 2 :], op=ALU.max)
            else:
                nc.vector.tensor_tensor(acc_all[:], acc_all[:], xt[:],
                                        op=ALU.max)

    HB = B // 2
    e2rep = fin_pool.tile([HB, NN], F32, tag="e2rep")

    def emit_e2_all():
        emit_e2()
        nc.gpsimd.partition_broadcast(e2rep[:], e2row[:])

    if stage1_ctx is not None:
        # timing scope "stage1": all prologue outside, loop over chunks only
        for q in range(0, NCHUNK, 4):
            tcasts_quad(q)
        for g in range(GROUPS):
            group_fw(g)
        emit_e2_all()
        with stage1_ctx:
            for c in range(NCHUNK):
                stage1(c)
    else:
        # group 0 prologue, then interleave next group's pieces with stage-1
        tcasts_quad(0)
        tcasts_quad(4)
        group_fw(0)
        for g in range(GROUPS):
            for ci in range(GCH):
                c = g * GCH + ci
                stage1(c)
                if c == 20:
                    emit_e2_all()
                if g + 1 < GROUPS:
                    if ci == 1:
                        tcasts_quad((g + 1) * GCH)
                    elif ci == 4:
                        tcasts_quad((g + 1) * GCH + 4)
                    elif ci == GCH - 1:
                        group_fw(g + 1)

    # ---------------- finals -----------------------------------------------
    accmax = acc_pool.tile([128, XW], BF16, tag="accmax")
    nc.gpsimd.partition_all_reduce(accmax[:], acc_all[:], 128, ReduceOp.max)
    for h in range(2):
        rows = slice(h * HB, (h + 1) * HB)
        mh = fin_pool.tile([HB, NN], BF16, tag=f"mh{h}")
        for j in range(HB):
            sec = h * HB + j
            eng = nc.sync if j % 2 == 0 else nc.scalar
            eng.dma_start(mh[j : j + 1, :],
                          accmax[0:1, sec * NN : (sec + 1) * NN])
        subh = fin_pool.tile([HB, NN], F32, tag=f"subh{h}")
        nc.vector.tensor_tensor(subh[:], e2rep[:], mh[:], op=ALU.subtract)
        nc.scalar.activation(subh[:], subh[:], AF.Exp, scale=-0.5)
        nc.vector.tensor_scalar(
            out=subh[:], in0=subh[:], scalar1=1.0, scalar2=None, op0=ALU.min)
        nc.sync.dma_start(out.ap()[rows, :], subh[:])


_NC_CACHE = None


def get_nc():
    global _NC_CACHE
    if _NC_CACHE is None:
        _NC_CACHE = build_bass()
    return _NC_CACHE


def make_in_maps(rel, arg1, arg2, fact_rel, fact_arg1, fact_arg2, entity_embeddings):
    n_per = F // 4

    def pad_fact(m):
        o = np.full((FP, E), 10.0, dtype=np.float32)
        o[:F] = m
        return o

    frp = pad_fact(fact_rel)
    f1p = pad_fact(fact_arg1)
    f2p = pad_fact(fact_arg2)
    relc = np.ascontiguousarray(rel, dtype=np.float32)
    a1c = np.ascontiguousarray(arg1, dtype=np.float32)
    a2c = np.ascontiguousarray(arg2, dtype=np.float32)
    in_maps = []
    for core in range(8):
        score, slot = (0, core) if core < 4 else (1, core - 4)
        ent_pad = np.zeros((NN, E), dtype=np.float32)
        ent_pad[:n_per] = entity_embeddings[slot * n_per : (slot + 1) * n_per]
        if score == 0:
            fc_, fw_, qw_ = f2p, f1p, a1c
        else:
            fc_, fw_, qw_ = f1p, f2p, a2c
        in_maps.append(
            {"f_c": fc_, "f_w": fw_, "f_rel": frp, "ent": ent_pad,
             "rel": relc, "qw": qw_}
        )
    return in_maps


def assemble(results):
    n_per = F // 4
    sp = np.concatenate([results[i]["out"][:, :n_per] for i in range(4)], axis=1)
    po = np.concatenate([results[i]["out"][:, :n_per] for i in range(4, 8)], axis=1)
    return sp.copy(), po.copy()


def kernel(rel, arg1, arg2, fact_rel, fact_arg1, fact_arg2, entity_embeddings):
    nc = get_nc()
    in_maps = make_in_maps(
        rel, arg1, arg2, fact_rel, fact_arg1, fact_arg2, entity_embeddings
    )
    res = bass_utils.run_bass_kernel_spmd(nc, in_maps, core_ids=list(range(8)))
    return assemble(res.results)

